# revision 32
# baseline (speedup 1.0000x reference)
"""Trainium2 Bass kernel for nn_AttentionTemporelle (3-window banded attention).

Reference computation (per batch element b):
    q = x @ Wq + bq ; k = x @ Wk + bk          [T, DK]
    s = q k^T / sqrt(DK)                        [T, T]
    acc = mean_w softmax(band_mask_w(s)) @ x    for w in (24, 168, 720)
    out = acc @ Wo + bo ; res = x + out ; LayerNorm(res) * gamma + beta

v3 design (vs the f32r baseline; 106.7us -> 88.2us TimelineSim):
  * x enters as bf16; PE transposes it per 128-tile in bf16 (1 cyc/row vs
    1.5 for f32r) and the psum->sbuf copies cast to fp8e4.
  * x-side projections qT/kT/xWo run as fp8 DoubleRow matmuls (0.5
    cyc/row, 256 contraction rows per instruction) - 12K PE cycles
    instead of the baseline's 52K for phase 0.
  * S computed per 128-row block over the 7-block halo strip in bf16;
    band-edge masking folded into the S PSUM accumulation as constant
    bf16 matmuls (corner tiles), so exp needs no separate mask pass.
  * One exp (ACT, accum -> Z720); two masked STTs give e168/e24 + their
    Z's; G assembled by two more STTs into a separate gmid tile; G
    transposed per-tile in bf16 (1 cyc/row) and PV runs bf16 against xWo.
  * LayerNorm: rowsum via the res-combine STT accum, sq via ACT Square
    accum; rstd computed on the DVE (int bit-trick + 2 Newton steps)
    because ACT Sqrt forces ~1.3us act-table reloads each way; finals in
    two batches (blocks 0-7 mid-kernel, 8-15 at the end), stores batched
    4 blocks per DMA, output bf16 (upcast on host).
  * Emission is software-pipelined 3 deep (S/exp of block i alongside
    G-transpose of i-1..i and PV/res of i-3), with the qT/kT/xWo jobs
    used as PE fillers; DMA loads split across the two HWDGE queues
    (per-queue DMAs serialize end-to-end).
  * Engine budget: DVE is the wall (the mask/assembly/res STT ops are
    DVE-only and get no 2x mode); PSUM->SBUF copies and sq sit on ACT,
    Pool only issues nothing (it cannot touch PSUM or run STT ops).
  * NOTE: dma_start_transpose (SBUF->SBUF or DRAM->SBUF) was tried for
    both the x transpose and the G transpose and produced
    NON-DETERMINISTIC results on hardware (FixedSemIncDMA: its
    completion semaphore increments +16, one per DMA engine, so
    consumers can observe partial data). Do not reintroduce it without
    solving that.

Sharding: pure data-parallel over B=8, one batch element per core.
"""

import math

import numpy as np

B, T, D, DK = 8, 2048, 512, 128
NBLK = T // 128                 # 16 row blocks
HALO = 3                        # 360 // 128 + 1 neighbor blocks each side
EPS = 1e-5
H720, H168, H24 = 360, 84, 12

WSC = 64.0                      # q/k weight prescale (fp8 range)
WOSC = 32.0                     # Wo prescale (fp8 range)
EXPSC = 1.0 / (WSC * WSC * math.sqrt(DK))
CNEG = -60.0 / EXPSC            # additive corner mask, pre-exp-scale units

_CACHE = {}


def _host_consts():
    import ml_dtypes

    bf = ml_dtypes.bfloat16
    r = np.arange(128)
    # inner-window masks, natural layout [t, 3*128 strip cols]
    c3 = np.arange(3 * 128)[None, :]
    d3 = (c3 - 128) - r[:, None]
    m168 = (np.abs(d3) <= H168).astype(bf)
    m24 = (np.abs(d3) <= H24).astype(bf)
    # corner masks as matmul lhsT consts: out[t, j] += lhsT[j, t]
    # lhsT[j, t] = CNEG if |d*128 + j - t| > 360 else 0 for d in (-3,-2,2,3)
    corn = np.zeros((128, 4, 128), dtype=np.float32)
    for idx, dlt in enumerate((-3, -2, 2, 3)):
        jj = r[:, None]
        tt = r[None, :]
        corn[:, idx, :] = np.where(
            np.abs(dlt * 128 + jj - tt) > H720, CNEG, 0.0
        )
    cornct = corn.astype(bf)
    identb = np.eye(128, dtype=np.float32).astype(bf)
    return m168, m24, cornct, identb


def _build_nc(has_bq, has_bk, has_bo, has_gamma, has_beta):
    import concourse.bass as bass
    import concourse.tile as tile
    from concourse import bacc, mybir

    f32 = mybir.dt.float32
    bf16 = mybir.dt.bfloat16
    f8 = mybir.dt.float8e4
    AF = mybir.ActivationFunctionType
    OP = mybir.AluOpType
    DR = mybir.MatmulPerfMode.DoubleRow

    nc = bacc.Bacc()

    xb_d = nc.declare_dram_parameter("xb", [T, D], bf16, isOutput=False)
    wq_d = nc.declare_dram_parameter("wq8", [128, 4, DK], f8, isOutput=False)
    wk_d = nc.declare_dram_parameter("wk8", [128, 4, DK], f8, isOutput=False)
    wo_d = nc.declare_dram_parameter("wo8", [128, 4, D], f8, isOutput=False)
    m168_d = nc.declare_dram_parameter("m168b", [128, 384], bf16, isOutput=False)
    m24_d = nc.declare_dram_parameter("m24b", [128, 384], bf16, isOutput=False)
    corn_d = nc.declare_dram_parameter("cornct", [128, 4, 128], bf16, isOutput=False)
    ident_d = nc.declare_dram_parameter("identb", [128, 128], bf16, isOutput=False)
    if has_bq:
        bq_d = nc.declare_dram_parameter("bq64", [DK, 1], f32, isOutput=False)
    if has_bk:
        bk_d = nc.declare_dram_parameter("bk64", [DK, 1], f32, isOutput=False)
    if has_gamma:
        gamma_d = nc.declare_dram_parameter("gamma_bc", [128, D], f32, isOutput=False)
    if has_beta:
        beta_d = nc.declare_dram_parameter("beta_bc", [128, D], f32, isOutput=False)
    out_d = nc.declare_dram_parameter("out", [T, D], bf16, isOutput=True)

    with tile.TileContext(nc) as tc:
        with tc.tile_pool(name="persist", bufs=1) as persist:
            xb_sb = persist.tile([128, NBLK, D], bf16, tag="xb")
            # fp8 transposed-x tiles per t-half: [d-part, chunk, t]
            xT8h = [
                persist.tile([128, 4, 1024], f8, tag=f"xT8{h}",
                             name=f"xT8{h}") for h in range(2)
            ]
            qT = persist.tile([128, T], bf16, tag="qT")
            kT = persist.tile([128, T], bf16, tag="kT")
            xWo = persist.tile([128, NBLK, D], bf16, tag="xWo")
            wq_sb = persist.tile([128, 4, DK], f8, tag="wq")
            wk_sb = persist.tile([128, 4, DK], f8, tag="wk")
            wo_sb = persist.tile([128, 4, D], f8, tag="wo")
            m168_sb = persist.tile([128, 384], bf16, tag="m168")
            m24_sb = persist.tile([128, 384], bf16, tag="m24")
            corn_sb = persist.tile([128, 4, 128], bf16, tag="corn")
            ident_sb = persist.tile([128, 128], bf16, tag="ident")
            eps_sb = persist.tile([128, 1], f32, tag="eps")
            nc.vector.memset(eps_sb, EPS)
            i32_ = mybir.dt.int32
            shift1_sb = persist.tile([128, 1], i32_, tag="sh1")
            nc.vector.memset(shift1_sb, 1)
            negone_sb = persist.tile([128, 1], i32_, tag="neg1")
            nc.vector.memset(negone_sb, -1)
            magic_sb_b = persist.tile([128, 16], i32_, tag="magic")
            nc.vector.memset(magic_sb_b, 0x5F3759DF)
            res16 = persist.tile([128, NBLK, D], bf16, tag="res16")
            out_sb = persist.tile([128, NBLK, D], bf16, tag="out_sb")
            rsA = persist.tile([128, NBLK], f32, tag="rsA")
            sq16 = persist.tile([128, NBLK], f32, tag="sq16")

            # ---------------- DMA loads ----------------
            # Per-queue DMAs serialize end-to-end: spread across the two
            # HWDGE queues (SP=sync, ACT=scalar), ordered by first use.
            x_r = xb_d[:].rearrange("(n p) d -> p n d", p=128)
            for qq in range(4):
                eng = [nc.sync, nc.scalar][qq % 2]
                eng.dma_start(
                    out=xb_sb[:, qq * 4:(qq + 1) * 4, :],
                    in_=x_r[:, qq * 4:(qq + 1) * 4, :],
                )
            nc.sync.dma_start(out=wq_sb, in_=wq_d[:])
            nc.scalar.dma_start(out=wk_sb, in_=wk_d[:])
            nc.sync.dma_start(out=corn_sb, in_=corn_d[:])
            nc.scalar.dma_start(out=ident_sb, in_=ident_d[:])
            nc.sync.dma_start(out=wo_sb, in_=wo_d[:])
            nc.scalar.dma_start(out=m168_sb, in_=m168_d[:])
            nc.sync.dma_start(out=m24_sb, in_=m24_d[:])
            if has_bq:
                bq_sb = persist.tile([128, 1], f32, tag="bq")
                nc.sync.dma_start(out=bq_sb, in_=bq_d[:])
            if has_bk:
                bk_sb = persist.tile([128, 1], f32, tag="bk")
                nc.sync.dma_start(out=bk_sb, in_=bk_d[:])
            if has_gamma:
                gamma_sb = persist.tile([128, D], f32, tag="gamma")
                nc.sync.dma_start(out=gamma_sb, in_=gamma_d[:])
            if has_beta:
                beta_sb = persist.tile([128, D], f32, tag="beta")
                nc.sync.dma_start(out=beta_sb, in_=beta_d[:])

            with (
                tc.tile_pool(name="s_ps", bufs=2, space="PSUM") as s_pool,
                tc.tile_pool(name="gt_ps", bufs=2, space="PSUM") as gt_pool,
                tc.tile_pool(name="big_ps", bufs=2, space="PSUM") as big_pool,
                tc.tile_pool(name="work", bufs=2) as work,
                tc.tile_pool(name="small", bufs=3) as small,
            ):
                # ---------- phase-0 jobs (PE fillers) ----------
                def ecopy(eng, out, in_):
                    if eng is nc.scalar:
                        nc.scalar.activation(out=out, in_=in_, func=AF.Copy)
                    else:
                        eng.tensor_copy(out=out, in_=in_)

                def cast_job(ti):
                    # transpose one t-block of x (4 d-chunks) on the PE and
                    # copy out as fp8 into the DoubleRow pair tiles
                    h, off = ti // 8, (ti % 8) * 128
                    xt_ps = gt_pool.tile([128, 512], bf16, tag="gt",
                                         name="xt_ps")
                    for c in range(4):
                        nc.tensor.matmul(
                            out=xt_ps[:, c * 128:(c + 1) * 128],
                            lhsT=xb_sb[:, ti, c * 128:(c + 1) * 128],
                            rhs=ident_sb,
                            is_transpose=True,
                            start=True, stop=True,
                        )
                    ecopy(
                        nc.scalar,
                        xT8h[h][:, :, off:off + 128],
                        xt_ps.rearrange("q (c t) -> q c t", c=4),
                    )

                # PSUM -> SBUF copies: DVE/ACT only (GPSIMD cannot read PSUM)
                copy_rr = [nc.scalar, nc.scalar]

                def qk_job(idx):
                    # idx: 0..7 -> (quarter, proj)
                    qtr, proj = idx // 2, idx % 2
                    w_sb = wq_sb if proj == 0 else wk_sb
                    dst = qT if proj == 0 else kT
                    ps = big_pool.tile([128, 512], f32, tag="big", name="qk_ps")
                    h, off = qtr // 2, (qtr % 2) * 512
                    for pr in range(2):
                        nc.tensor.matmul(
                            out=ps,
                            lhsT=w_sb[:, 2 * pr:2 * pr + 2, :],
                            rhs=xT8h[h][:, 2 * pr:2 * pr + 2, off:off + 512],
                            start=(pr == 0),
                            stop=(pr == 1),
                            perf_mode=DR,
                        )
                    bias = None
                    if proj == 0 and has_bq:
                        bias = bq_sb
                    if proj == 1 and has_bk:
                        bias = bk_sb
                    if bias is not None:
                        nc.scalar.activation(
                            out=dst[:, qtr * 512:(qtr + 1) * 512],
                            in_=ps, func=AF.Identity, bias=bias, scale=1.0,
                        )
                    else:
                        ecopy(copy_rr[idx % 2],
                              dst[:, qtr * 512:(qtr + 1) * 512], ps)

                def xwo_job(j):
                    ps = big_pool.tile([128, 512], f32, tag="big", name="xwo_ps")
                    h, off = j // 8, (j % 8) * 128
                    for pr in range(2):
                        nc.tensor.matmul(
                            out=ps,
                            lhsT=xT8h[h][:, 2 * pr:2 * pr + 2, off:off + 128],
                            rhs=wo_sb[:, 2 * pr:2 * pr + 2, :],
                            start=(pr == 0),
                            stop=(pr == 1),
                            perf_mode=DR,
                        )
                    ecopy(copy_rr[j % 2], xWo[:, j, :], ps)

                qk_done = [False] * 8
                xwo_done = [False] * NBLK

                def ensure_qk(qtr):
                    for idx in (qtr * 2, qtr * 2 + 1):
                        if not qk_done[idx]:
                            qk_job(idx)
                            qk_done[idx] = True

                def ensure_xwo(j):
                    if not xwo_done[j]:
                        xwo_job(j)
                        xwo_done[j] = True

                def filler(n):
                    k = 0
                    for idx in range(8):
                        if k >= n:
                            return
                        if not qk_done[idx]:
                            qk_job(idx)
                            qk_done[idx] = True
                            k += 1
                    for j in range(NBLK):
                        if k >= n:
                            return
                        if not xwo_done[j]:
                            xwo_job(j)
                            xwo_done[j] = True
                            k += 1

                # ---------- prologue ----------
                for ti in range(4):
                    cast_job(ti)
                ensure_qk(0)
                for j in range(4):
                    ensure_xwo(j)
                for ti in range(4, 8):
                    cast_job(ti)
                ensure_qk(1)
                for ti in range(8, 16):
                    cast_job(ti)

                # ---------- per-block state ----------
                st = [dict() for _ in range(NBLK)]

                def geom(i):
                    jlo, jhi = max(0, i - HALO), min(NBLK - 1, i + HALO)
                    mlo, mhi = max(0, i - 1), min(NBLK - 1, i + 1)
                    return jlo, jhi, mlo, mhi

                def emit_S(i):
                    jlo, jhi, _, _ = geom(i)
                    ensure_qk(min(jhi, NBLK - 1) // 4)
                    s_ps = s_pool.tile([128, 896], f32, tag="s", name="s_ps")
                    st[i]["s_ps"] = s_ps
                    cidx = {-3: 0, -2: 1, 2: 2, 3: 3}
                    for jb in range(jlo, jhi + 1):
                        sl = jb - jlo
                        dlt = jb - i
                        corner = abs(dlt) >= 2
                        nc.tensor.matmul(
                            out=s_ps[:, sl * 128:(sl + 1) * 128],
                            lhsT=qT[:, i * 128:(i + 1) * 128],
                            rhs=kT[:, jb * 128:(jb + 1) * 128],
                            start=True,
                            stop=not corner,
                        )
                        if corner:
                            nc.tensor.matmul(
                                out=s_ps[:, sl * 128:(sl + 1) * 128],
                                lhsT=corn_sb[:, cidx[dlt], :],
                                rhs=ident_sb,
                                start=False,
                                stop=True,
                            )

                def emit_exp(i):
                    jlo, jhi, _, _ = geom(i)
                    ncols = (jhi - jlo + 1) * 128
                    em = work.tile([128, 896], bf16, tag="em", bufs=4, name="em")
                    z3 = small.tile([128, 3], f32, tag="z3", name="z3")
                    st[i]["em"] = em
                    st[i]["z3"] = z3
                    nc.scalar.activation(
                        out=em[:, :ncols],
                        in_=st[i]["s_ps"][:, :ncols],
                        func=AF.Exp,
                        scale=EXPSC,
                        accum_out=z3[:, 0:1],
                    )

                def emit_vec(i):
                    # masked inner windows + normalizers + G-mid assembly
                    jlo, jhi, mlo, mhi = geom(i)
                    mcols = (mhi - mlo + 1) * 128
                    ms = (mlo - jlo) * 128
                    mc = (mlo - (i - 1)) * 128
                    em = st[i]["em"]
                    z3 = st[i]["z3"]
                    e168 = work.tile([128, 384], bf16, tag="e168", name="e168")
                    e24 = work.tile([128, 384], bf16, tag="e24", name="e24")
                    nc.vector.scalar_tensor_tensor(
                        out=e168[:, :mcols],
                        in0=em[:, ms:ms + mcols],
                        scalar=1.0,
                        in1=m168_sb[:, mc:mc + mcols],
                        op0=OP.mult, op1=OP.mult,
                        accum_out=z3[:, 1:2],
                    )
                    nc.vector.scalar_tensor_tensor(
                        out=e24[:, :mcols],
                        in0=em[:, ms:ms + mcols],
                        scalar=1.0,
                        in1=m24_sb[:, mc:mc + mcols],
                        op0=OP.mult, op1=OP.mult,
                        accum_out=z3[:, 2:3],
                    )
                    rcp = small.tile([128, 3], f32, tag="rcp", name="rcp")
                    nc.vector.reciprocal(out=rcp, in_=z3)
                    st[i]["rcp"] = rcp
                    cc = small.tile([128, 2], f32, tag="cc", name="cc")
                    nc.vector.tensor_scalar(
                        out=cc, in0=rcp[:, 1:3], scalar1=z3[:, 0:1],
                        scalar2=None, op0=OP.mult,
                    )
                    sc = small.tile([128, 1], f32, tag="sc", name="sc")
                    nc.vector.tensor_scalar_mul(
                        out=sc, in0=rcp[:, 0:1], scalar1=1.0 / WOSC
                    )
                    st[i]["sc"] = sc
                    # G-mid into separate tile (avoid write-after-read
                    # hazards on em against the transpose reads)
                    gmid = work.tile([128, 384], bf16, tag="gmid", bufs=3,
                                     name="gmid")
                    st[i]["gmid"] = gmid
                    nc.vector.scalar_tensor_tensor(
                        out=gmid[:, :mcols],
                        in0=e168[:, :mcols],
                        scalar=cc[:, 0:1],
                        in1=em[:, ms:ms + mcols],
                        op0=OP.mult, op1=OP.add,
                    )
                    nc.vector.scalar_tensor_tensor(
                        out=gmid[:, :mcols],
                        in0=e24[:, :mcols],
                        scalar=cc[:, 1:2],
                        in1=gmid[:, :mcols],
                        op0=OP.mult, op1=OP.add,
                    )

                def emit_gts(i):
                    # PE transposes of G (em updated in place) + copies out
                    jlo, jhi, mlo, mhi = geom(i)
                    nb = jhi - jlo + 1
                    ncols = nb * 128
                    em = st[i]["em"]
                    gt = gt_pool.tile([128, 896], bf16, tag="gt", name="gt_ps")
                    gmid = st[i]["gmid"]
                    for sl in range(nb):
                        jb = jlo + sl
                        if mlo <= jb <= mhi:
                            mo = (jb - mlo) * 128
                            lhsT = gmid[:, mo:mo + 128]
                        else:
                            lhsT = em[:, sl * 128:(sl + 1) * 128]
                        nc.tensor.matmul(
                            out=gt[:, sl * 128:(sl + 1) * 128],
                            lhsT=lhsT,
                            rhs=ident_sb,
                            is_transpose=True,
                            start=True, stop=True,
                        )
                    gts = work.tile([128, 7, 128], bf16, tag="gts", bufs=4,
                                    name="gts")
                    st[i]["gts"] = gts
                    n1 = min(ncols, 512)
                    nc.vector.tensor_copy(
                        out=gts.rearrange("p a b -> p (a b)")[:, :n1],
                        in_=gt[:, :n1],
                    )
                    if ncols > n1:
                        nc.scalar.activation(
                            out=gts.rearrange("p a b -> p (a b)")[:, n1:ncols],
                            in_=gt[:, n1:ncols],
                            func=AF.Copy,
                        )

                def emit_PV(i):
                    jlo, jhi, _, _ = geom(i)
                    nb = jhi - jlo + 1
                    ensure_xwo(min(jhi, NBLK - 1))
                    gts = st[i]["gts"]
                    pv = big_pool.tile([128, 512], f32, tag="big", name="pv_ps")
                    st[i]["pv"] = pv
                    for sl in range(nb):
                        nc.tensor.matmul(
                            out=pv,
                            lhsT=gts[:, sl, :],
                            rhs=xWo[:, jlo + sl, :],
                            start=(sl == 0),
                            stop=(sl == nb - 1),
                        )

                def emit_res(i):
                    pv = st[i]["pv"]
                    sc = st[i]["sc"]
                    nc.vector.scalar_tensor_tensor(
                        out=res16[:, i, :],
                        in0=pv,
                        scalar=sc[:, 0:1],
                        in1=xb_sb[:, i, :],
                        op0=OP.mult, op1=OP.add,
                        accum_out=rsA[:, i:i + 1],
                    )
                    sqscr = work.tile([128, D], bf16, tag="sqscr", name="sqscr")
                    nc.scalar.activation(
                        out=sqscr,
                        in_=res16[:, i, :],
                        func=AF.Square,
                        accum_out=sq16[:, i:i + 1],
                    )
                    st[i].clear()

                def ln_tail(h0, hn):
                    hsl = slice(h0, h0 + hn)
                    mu = small.tile([128, hn], f32, tag="mu", name="mu")
                    var = small.tile([128, hn], f32, tag="var", name="var")
                    nc.vector.tensor_scalar_mul(
                        out=mu, in0=rsA[:, hsl], scalar1=1.0 / D
                    )
                    nc.vector.tensor_scalar_mul(
                        out=var, in0=sq16[:, hsl], scalar1=1.0 / D
                    )
                    musq = small.tile([128, hn], f32, tag="musq", name="musq")
                    nc.vector.tensor_mul(out=musq, in0=mu, in1=mu)
                    nc.vector.tensor_sub(out=var, in0=var, in1=musq)
                    # rstd = rsqrt(var + eps) on DVE: int bit-trick seed +
                    # 2 Newton steps (ACT Sqrt would thrash the act table)
                    nc.vector.tensor_scalar(
                        out=var, in0=var, scalar1=1.0, scalar2=EPS,
                        op0=OP.mult, op1=OP.add,
                    )
                    i32 = mybir.dt.int32
                    rstd = small.tile([128, hn], f32, tag="rstd", name="rstd")
                    nc.vector.tensor_scalar(
                        out=rstd.bitcast(i32), in0=var.bitcast(i32),
                        scalar1=shift1_sb[:, 0:1], scalar2=None,
                        op0=OP.logical_shift_right,
                    )
                    nc.vector.scalar_tensor_tensor(
                        out=rstd.bitcast(i32), in0=rstd.bitcast(i32),
                        scalar=negone_sb[:, 0:1], in1=magic_sb_b[:, 0:hn],
                        op0=OP.mult, op1=OP.add,
                    )
                    nt = small.tile([128, hn], f32, tag="nt", name="nt")
                    for _ in range(2):
                        nc.vector.tensor_mul(out=nt, in0=rstd, in1=rstd)
                        nc.vector.tensor_mul(out=nt, in0=nt, in1=var)
                        nc.vector.tensor_scalar(
                            out=nt, in0=nt, scalar1=-0.5, scalar2=1.5,
                            op0=OP.mult, op1=OP.add,
                        )
                        nc.vector.tensor_mul(out=rstd, in0=rstd, in1=nt)
                    nmb = small.tile([128, hn], f32, tag="nmb", name="nmb")
                    nc.vector.tensor_mul(out=nmb, in0=mu, in1=rstd)
                    nc.vector.tensor_scalar_mul(out=nmb, in0=nmb, scalar1=-1.0)
                    for k in range(hn):
                        ib = h0 + k
                        ow = out_sb[:, ib, :]
                        nc.vector.tensor_scalar(
                            out=ow,
                            in0=res16[:, ib, :],
                            scalar1=rstd[:, k:k + 1],
                            scalar2=nmb[:, k:k + 1],
                            op0=OP.mult,
                            op1=OP.add,
                        )
                        if has_gamma:
                            nc.gpsimd.tensor_mul(
                                out=ow, in0=ow, in1=gamma_sb
                            )
                        if has_beta:
                            nc.gpsimd.tensor_add(
                                out=ow, in0=ow, in1=beta_sb
                            )
                        if ib % 4 == 3:
                            out_r = out_d[:].rearrange(
                                "(n p) d -> p n d", p=128
                            )
                            g = ib // 4
                            eng = [nc.scalar, nc.sync][g % 2]
                            eng.dma_start(
                                out=out_r[:, g * 4:(g + 1) * 4, :],
                                in_=out_sb[:, g * 4:(g + 1) * 4, :],
                            )

                # ---------- pipelined emission (lookahead-3) ----------
                for i in range(NBLK):
                    emit_S(i)
                    emit_exp(i)
                    if i >= 3:
                        emit_PV(i - 3)
                    filler(1)
                    emit_vec(i)
                    emit_gts(i)
                    if i >= 3:
                        emit_res(i - 3)
                    if i == 11:
                        ln_tail(0, 8)
                for j in (NBLK - 3, NBLK - 2, NBLK - 1):
                    filler(100)
                    emit_PV(j)
                    emit_res(j)
                ln_tail(8, 8)

    nc.compile()
    return nc


def _get_built(flags):
    if flags not in _CACHE:
        _CACHE[flags] = _build_nc(*flags)
    return _CACHE[flags]


def _make_in_maps(x, Wq, bq, Wk, bk, Wo, bo, gamma, beta, flags):
    import ml_dtypes

    bf = ml_dtypes.bfloat16
    f8 = ml_dtypes.float8_e4m3
    has_bq, has_bk, has_bo, has_gamma, has_beta = flags
    m168, m24, cornct, identb = _host_consts()
    base = {
        "wq8": np.ascontiguousarray(
            (Wq * WSC).reshape(4, 128, DK).transpose(1, 0, 2).astype(f8)
        ),
        "wk8": np.ascontiguousarray(
            (Wk * WSC).reshape(4, 128, DK).transpose(1, 0, 2).astype(f8)
        ),
        "wo8": np.ascontiguousarray(
            (Wo * (WOSC / 3.0)).reshape(4, 128, D).transpose(1, 0, 2).astype(f8)
        ),
        "m168b": m168,
        "m24b": m24,
        "cornct": cornct,
        "identb": identb,
    }
    if has_bq:
        base["bq64"] = np.ascontiguousarray(bq * WSC, dtype=np.float32).reshape(DK, 1)
    if has_bk:
        base["bk64"] = np.ascontiguousarray(bk * WSC, dtype=np.float32).reshape(DK, 1)
    if has_gamma:
        base["gamma_bc"] = np.broadcast_to(
            np.asarray(gamma, dtype=np.float32), (128, D)
        ).copy()
    if has_beta:
        base["beta_bc"] = np.broadcast_to(
            np.asarray(beta, dtype=np.float32), (128, D)
        ).copy()
    xh = x if not has_bo else x + np.asarray(bo, dtype=np.float32)[None, None, :]
    return [
        {**base, "xb": np.ascontiguousarray(xh[core]).astype(bf)}
        for core in range(B)
    ]


def kernel(x, Wq, bq, Wk, bk, Wo, bo, gamma, beta):
    from concourse.bass_utils import run_bass_kernel_spmd

    x = np.asarray(x, dtype=np.float32)
    Wq = np.asarray(Wq, dtype=np.float32)
    bq = np.asarray(bq, dtype=np.float32)
    Wk = np.asarray(Wk, dtype=np.float32)
    bk = np.asarray(bk, dtype=np.float32)
    Wo = np.asarray(Wo, dtype=np.float32)
    bo = np.asarray(bo, dtype=np.float32)
    gamma = np.asarray(gamma, dtype=np.float32)
    beta = np.asarray(beta, dtype=np.float32)

    flags = (
        bool(np.any(bq != 0.0)),
        bool(np.any(bk != 0.0)),
        bool(np.any(bo != 0.0)),
        bool(np.any(gamma != 1.0)),
        bool(np.any(beta != 0.0)),
    )
    nc = _get_built(flags)
    in_maps = _make_in_maps(x, Wq, bq, Wk, bk, Wo, bo, gamma, beta, flags)
    res = run_bass_kernel_spmd(nc, in_maps, list(range(B)))
    return np.stack([np.asarray(res.results[c]["out"]).astype(np.float32) for c in range(B)], axis=0)
